# revision 10
# baseline (speedup 1.0000x reference)
"""Causal self-attention (B=4, S=2048, E=1024, H=16, D=64) on 8 TRN2 NeuronCores.

Sharding: hybrid data/tensor parallel.  Core c handles batch b = c // 2 and
head group hg = c % 2 (8 of 16 heads).  Each core computes its batch's full
attention for its heads plus the partial output projection; the host sums the
two head-group partials per batch (the "all-reduce") and adds out_b.

Per-core pipeline (all matmuls in float32r: full fp32 storage, ~1 cyc/row):
  1. QKV projection from x^T (host-pretransposed):
       qT/kT produced head-PAIR-packed [128 = (h_even d64 | h_odd d64), S]
       v produced in natural [token, (h,d)] layout with a ones column
       appended per head (v') so that PV accumulates the softmax denominator
       as row 64 ("ones trick").
  2. Attention, logits TRANSPOSED (lT[k, q] tiles [128, 512]) so softmax
     reduction lands on the matmul contraction instead of partitions:
       QK row-packed two heads per array pass (bases 0/64),
       causal: lower-triangle k-tiles only; diagonal tiles get a mask add,
       exp on ACT (no max-subtraction: logits are O(1) after folding
       1/sqrt(D) into wq on the host), PV accumulates [65, 512] per head.
  3. Normalize: rz = 1/Z (DVE), PE broadcast (K=1 matmul), DVE mul ->
     attnT [128 = head pair, S] per pair.
  4. Output projection: out[t, e] accumulated over the 4 head-pair tiles.

The staged walrus build only accepts ONE sync-wait per instruction; a BIR
post-pass hoists extra waits onto same-engine NoOps (program order on the
sequencer preserves semantics).  The Tile tail drain is rebuilt the same way.
NEFFs are disk-cached keyed on the BIR hash.
"""

import hashlib
import os
import numpy as np

import concourse.bass as bass
import concourse.tile as tile
from concourse import mybir

B, S, E, H, D = 4, 2048, 1024, 16, 64
N_CORES = 8
HPC = H // 2          # heads per core (8)
NPAIR = HPC // 2      # head pairs per core (4)
QC = 512              # q-chunk (free dim per matmul)
NQC = S // QC         # 4 q-chunks
KT = 128              # k-tile
NKT = S // KT         # 16 k-tiles
NTT = S // 128        # 16 token tiles
ET = 128              # e-tile (contraction)
NET = E // ET         # 8 e-tiles

f32 = mybir.dt.float32
f32r = mybir.dt.float32r
AF = mybir.ActivationFunctionType

MASK_VAL = -1e30

# ---------------------------------------------------------------------------
# compile workarounds + NEFF cache
# ---------------------------------------------------------------------------

_NEFF_CACHE_DIR = os.path.join(
    os.environ.get("XDG_CACHE_HOME", os.path.expanduser("~/.cache")), "bass_neff_cache"
)

_patched = [False]


def _apply_patches():
    if _patched[0]:
        return
    _patched[0] = True
    import orjson
    import bass_rust as _br
    import concourse.bass_utils as bass_utils
    import concourse.bass2jax as bass2jax

    # --- 1. split multi-wait instructions in the BIR ---
    counter = [0]

    def _split(m):
        for func in m.get("functions", []):
            for bb in func.get("blocks", []):
                new_insts = []
                for inst in bb.get("instructions", []):
                    si = inst.get("sync_info")
                    waits = si.get("on_wait") if si else None
                    if waits and len(waits) > 1:
                        for w in waits[:-1]:
                            counter[0] += 1
                            new_insts.append({
                                "debug": 0,
                                "engine": inst.get("engine"),
                                "ins": [],
                                "name": f"WSPLIT-{counter[0]}",
                                "opcode": "NoOp",
                                "outs": [],
                                "sync_info": {"on_update": [], "on_wait": [w]},
                            })
                        si["on_wait"] = [waits[-1]]
                    new_insts.append(inst)
                bb["instructions"] = new_insts

    orig_compile = bass_utils.compile_bir_kernel

    def compile_bir_kernel_split(bir_json, tmpdir, neff_name="file.neff", **kw):
        if isinstance(bir_json, str):
            bir_json = bir_json.encode()
        m = orjson.loads(bir_json)
        _split(m)
        return orig_compile(orjson.dumps(m), tmpdir, neff_name=neff_name, **kw)

    bass_utils.compile_bir_kernel = compile_bir_kernel_split
    bass2jax.compile_bir_kernel = compile_bir_kernel_split

    # --- 2. tail drain: one wait per SP nop ---
    def _drain_and_barrier(self, tick_clock, wait_clock):
        gc = tick_clock.global_clock
        n_procs = _br.N_PROCS
        ticks = [gc[p] for p in range(n_procs)]
        for p in range(n_procs):
            if ticks[p] <= 0:
                continue
            vals = [ticks[q] if q == p else 0 for q in range(n_procs)]
            nop_inst = self.nc.sync.nop(nofuse=True, hint="drain_wait_split")
            wait_clock.add_sem_waits(
                nop_inst.ins, _br.ScopedClock({None: _br.VectorClock(vals)})
            )
        self.nc.sync.drain()
        self.nc.all_engine_barrier()
        assert self.sems is not None
        popped = self.nc._tile_sem_poison_stack.pop()
        assert popped is self._sem_poison
        self.nc.clear_and_free_semaphores(list(self.sems.allocated().values()))
        self.nc.all_engine_barrier()

    tile.TileContext._drain_and_barrier = _drain_and_barrier

    # --- 3. NEFF disk cache keyed on (split) BIR bytes ---
    import libneuronxla  # noqa: F401

    bass2jax.install_neuronx_cc_hook()
    hooked = libneuronxla.neuronx_cc

    def cached_neuronx_cc(code, code_format, platform_version, file_prefix):
        key = hashlib.sha256(
            code + bytes(str(platform_version), "utf8")
        ).hexdigest()
        path = os.path.join(_NEFF_CACHE_DIR, key + ".bin")
        if os.path.exists(path):
            with open(path, "rb") as f:
                return 0, f.read()
        ret, data = hooked(code, code_format, platform_version, file_prefix)
        if ret == 0 and isinstance(data, (bytes, bytearray)):
            os.makedirs(_NEFF_CACHE_DIR, exist_ok=True)
            tmp = path + f".tmp{os.getpid()}"
            with open(tmp, "wb") as f:
                f.write(data)
            os.replace(tmp, path)
        return ret, data

    libneuronxla.neuronx_cc = cached_neuronx_cc
    # keep install_neuronx_cc_hook (called by others) from undoing the wrap
    bass2jax.install_neuronx_cc_hook = lambda: None


# ---------------------------------------------------------------------------
# kernel builder
# ---------------------------------------------------------------------------

def _build_nc(with_qkv_bias: bool):
    nc = bass.Bass()
    bf16 = mybir.dt.bfloat16

    xT = nc.declare_dram_parameter("xT", [E, S], bf16, isOutput=False)
    wqk = nc.declare_dram_parameter("wqk", [128, NET * 8 * 128], bf16, isOutput=False)
    wv = nc.declare_dram_parameter("wv", [128, NET * 512], bf16, isOutput=False)
    ow = nc.declare_dram_parameter("ow", [128, NPAIR * 1024], bf16, isOutput=False)
    masks = nc.declare_dram_parameter("masks", [128, 4 * 2 * QC], f32, isOutput=False)
    cones = nc.declare_dram_parameter("cones", [128, 128], bf16, isOutput=False)
    selq = nc.declare_dram_parameter("selq", [128, 4 * 64], bf16, isOutput=False)
    if with_qkv_bias:
        qb = nc.declare_dram_parameter("qb", [1, 8 * 128], bf16, isOutput=False)
        vb = nc.declare_dram_parameter("vb", [1, 512], bf16, isOutput=False)
    out = nc.declare_dram_parameter("out", [S, E], f32, isOutput=True)

    with tile.TileContext(nc) as tc, nc.allow_low_precision(
        reason="bf16 matmul inputs; accumulation stays fp32 in PSUM"
    ):
        with (
            tc.tile_pool(name="sbP", bufs=1) as sbP,
            tc.tile_pool(name="sbX", bufs=3) as sbX,
            tc.tile_pool(name="sbT", bufs=6) as sbT,
            tc.tile_pool(name="psG", bufs=2, space="PSUM") as psG,
            tc.tile_pool(name="psL", bufs=2, space="PSUM") as psL,
            tc.tile_pool(name="psAt", bufs=1, space="PSUM") as psAt,
        ):
            qT2 = sbP.tile([128, NPAIR * S], bf16, tag="qT2")
            kT2 = sbP.tile([128, NPAIR * S], bf16, tag="kT2")
            vsb = sbP.tile([128, NTT * HPC, 65], bf16, tag="vsb")
            attnT = sbP.tile([128, NPAIR * S], bf16, tag="attnT")
            masks_sb = sbP.tile([128, 4 * 2 * QC], f32, tag="masks")
            cones_sb = sbP.tile([128, 128], bf16, tag="cones")
            selq_sb = sbP.tile([128, 4 * 64], bf16, tag="selq")
            wqk_sb = sbP.tile([128, NET * 8 * 128], bf16, tag="wqk")
            wv_sb = sbP.tile([128, NET * 512], bf16, tag="wv")
            ow_sb = sbP.tile([128, NPAIR * 1024], bf16, tag="ow")
            # chunked weight loads so the first matmuls start early
            for et in range(NET):
                nc.sync.dma_start(
                    wqk_sb[:, et * 1024:(et + 1) * 1024],
                    wqk[:, et * 1024:(et + 1) * 1024],
                )
            for et in range(NET):
                nc.sync.dma_start(
                    wv_sb[:, et * 512:(et + 1) * 512],
                    wv[:, et * 512:(et + 1) * 512],
                )
            nc.sync.dma_start(cones_sb[:], cones[:])
            nc.sync.dma_start(selq_sb[:], selq[:])
            nc.sync.dma_start(masks_sb[:], masks[:])
            nc.sync.dma_start(ow_sb[:], ow[:])
            nc.sync.dma_start(vsb[:, :, 64:65], cones[:, 0:128])
            if with_qkv_bias:
                qb_sb = sbP.tile([1, 8 * 128], bf16, tag="qb")
                vb_sb = sbP.tile([1, 512], bf16, tag="vb")
                nc.sync.dma_start(qb_sb[:], qb[:])
                nc.sync.dma_start(vb_sb[:], vb[:])

            def emit_proj(qc):
                xc = sbX.tile([128, NET, QC], bf16, tag="xc", name="xc")
                for et in range(NET):
                    nc.sync.dma_start(
                        xc[:, et, :],
                        xT[et * 128:(et + 1) * 128, qc * QC:(qc + 1) * QC],
                    )
                for m in range(8):
                    ps = psG.tile([128, QC], f32, tag="gen", name="qkps")
                    if with_qkv_bias:
                        nc.tensor.matmul(
                            ps[:], qb_sb[:, m * 128:(m + 1) * 128],
                            cones_sb[0:1, 0:128], start=True, stop=False,
                        )
                    for et in range(NET):
                        nc.tensor.matmul(
                            ps[:],
                            wqk_sb[:, (et * 8 + m) * 128:(et * 8 + m + 1) * 128],
                            xc[:, et, :],
                            start=(et == 0 and not with_qkv_bias),
                            stop=(et == NET - 1),
                        )
                    dst = qT2 if m < NPAIR else kT2
                    blk = m if m < NPAIR else m - NPAIR
                    nc.vector.tensor_copy(
                        dst[:, blk * S + qc * QC: blk * S + (qc + 1) * QC], ps[:]
                    )
                for tl in range(QC // 128):
                    tt = qc * (QC // 128) + tl
                    psv = psG.tile([128, 512], f32, tag="gen", name="vps")
                    if with_qkv_bias:
                        nc.tensor.matmul(
                            psv[:], cones_sb[0:1, 0:128], vb_sb[:],
                            start=True, stop=False,
                        )
                    for et in range(NET):
                        nc.tensor.matmul(
                            psv[:],
                            xc[:, et, tl * 128:(tl + 1) * 128],
                            wv_sb[:, et * 512:(et + 1) * 512],
                            start=(et == 0 and not with_qkv_bias),
                            stop=(et == NET - 1),
                        )
                    nc.vector.tensor_copy(
                        vsb[:, tt * HPC:(tt + 1) * HPC, 0:64],
                        psv[:].rearrange("p (h d) -> p h d", h=HPC),
                    )

            def emit_qkpv(qc):
                n_kt = 4 * qc + 4
                z4 = []
                for _ in range(2):
                    zt = sbT.tile([128, QC], f32, tag="z4", name="z4", bufs=4)
                    nc.vector.memset(zt[:], 1.0)
                    z4.append(zt)
                nms = [None] * 8
                for hp in range(NPAIR):
                    at = [None, None]
                    for r in range(2):
                        at[r] = psAt.tile([65, QC], f32, tag=f"at{r}", name=f"at{r}")
                    for i in range(n_kt):
                        lt = psL.tile([128, 2 * QC], f32, tag="lt", name="lt")
                        for r in range(2):
                            nc.tensor.matmul(
                                lt[:, r * QC:(r + 1) * QC],
                                kT2[r * 64:(r + 1) * 64,
                                    hp * S + i * KT: hp * S + (i + 1) * KT],
                                qT2[r * 64:(r + 1) * 64,
                                    hp * S + qc * QC: hp * S + (qc + 1) * QC],
                                start=True, stop=True,
                            )
                        j = i - 4 * qc
                        if j >= 0:
                            nc.vector.tensor_add(
                                lt[:], lt[:],
                                masks_sb[:, j * 2 * QC:(j + 1) * 2 * QC],
                            )
                        p2 = sbT.tile([128, 2 * QC], bf16, tag="p2", name="p2")
                        nc.scalar.activation(out=p2[:], in_=lt[:], func=AF.Exp)
                        for r in range(2):
                            h = 2 * hp + r
                            nc.tensor.matmul(
                                at[r][:],
                                vsb[:, i * HPC + h, :],
                                p2[:, r * QC:(r + 1) * QC],
                                start=(i == 0), stop=(i == n_kt - 1),
                            )
                    for r in range(2):
                        idx = 2 * hp + r
                        a, jj = divmod(idx, 4)
                        nm = sbT.tile([64, QC], f32, tag="nm", name="nm", bufs=10)
                        nc.scalar.copy(nm[:], at[r][0:64, :])
                        nc.vector.tensor_copy(
                            z4[a][32 * jj: 32 * jj + 1, :], at[r][64:65, :]
                        )
                        nms[idx] = nm
                rz4b = []
                for a in range(2):
                    rz = sbT.tile([128, QC], f32, tag="rz4", name="rz4", bufs=4)
                    nc.vector.reciprocal(rz[:], z4[a][:])
                    rzb = sbT.tile([128, QC], bf16, tag="rz4b", name="rz4b", bufs=4)
                    nc.vector.tensor_copy(rzb[:], rz[:])
                    rz4b.append(rzb)
                return nms, rz4b

            def emit_norm_out(qc, state):
                nms, rz4b = state
                for hp in range(NPAIR):
                    for r in range(2):
                        idx = 2 * hp + r
                        a, jj = divmod(idx, 4)
                        bc = psG.tile([64, QC], f32, tag="gen", name="bc")
                        nc.tensor.matmul(
                            bc[:], selq_sb[:, jj * 64:(jj + 1) * 64], rz4b[a][:],
                            start=True, stop=True,
                        )
                        nc.vector.tensor_mul(
                            attnT[r * 64:(r + 1) * 64,
                                  hp * S + qc * QC: hp * S + (qc + 1) * QC],
                            nms[idx][:], bc[:],
                        )
                for tl in range(QC // 128):
                    tt = qc * (QC // 128) + tl
                    for ec in range(2):
                        po = psG.tile([128, 512], f32, tag="gen", name="outps")
                        for hp in range(NPAIR):
                            nc.tensor.matmul(
                                po[:],
                                attnT[:, hp * S + tt * 128: hp * S + (tt + 1) * 128],
                                ow_sb[:, hp * 1024 + ec * 512: hp * 1024 + (ec + 1) * 512],
                                start=(hp == 0), stop=(hp == NPAIR - 1),
                            )
                        o_sb = sbT.tile([128, 512], f32, tag="osb", name="osb")
                        nc.vector.tensor_copy(o_sb[:], po[:])
                        nc.sync.dma_start(
                            out[tt * 128:(tt + 1) * 128, ec * 512:(ec + 1) * 512],
                            o_sb[:],
                        )

            # software-pipelined emission: proj(qc+1) fills the normalize
            # latency of chunk qc on the in-order PE stream
            emit_proj(0)
            state = emit_qkpv(0)
            for qc in range(1, NQC):
                emit_proj(qc)
                emit_norm_out(qc - 1, state)
                state = emit_qkpv(qc)
            emit_norm_out(NQC - 1, state)
    return nc


# ---------------------------------------------------------------------------
# host-side packing
# ---------------------------------------------------------------------------

def _pack_inputs(x, qkv_w, qkv_b, out_w):
    """Returns (in_maps list of 8 dicts, with_qkv_bias)."""
    import ml_dtypes
    bf16 = ml_dtypes.bfloat16
    x = np.asarray(x, np.float32)
    qkv_w = np.asarray(qkv_w, np.float32)
    qkv_b = np.asarray(qkv_b, np.float32)
    out_w = np.asarray(out_w, np.float32)
    with_bias = bool(np.any(qkv_b))

    scale = 1.0 / np.sqrt(D)
    xT_b = [np.ascontiguousarray(x[b].T).astype(bf16) for b in range(B)]

    # causal diagonal masks for the merged two-head tile:
    # [128, j, 2, 512] with the same mask in both halves
    kl = np.arange(128)[:, None]
    ql = np.arange(QC)[None, :]
    masks_np = np.empty((128, 4, 2, QC), np.float32)
    for j in range(4):
        mj = np.where(kl + 128 * j <= ql, 0.0, MASK_VAL).astype(np.float32)
        masks_np[:, j, 0, :] = mj
        masks_np[:, j, 1, :] = mj
    masks_np = masks_np.reshape(128, -1)
    cones_np = np.ones((128, 128), bf16)
    selq_np = np.zeros((128, 4 * 64), np.float32)
    for jj in range(4):
        selq_np[32 * jj, jj * 64:(jj + 1) * 64] = 1.0
    selq_np = selq_np.astype(bf16)

    in_maps = []
    for c in range(N_CORES):
        b, hg = divmod(c, 2)
        hs = hg * HPC
        heads = list(range(hs, hs + HPC))

        wqk_np = np.empty((E, 8, 128), np.float32)
        for hp in range(NPAIR):
            h0, h1 = hs + 2 * hp, hs + 2 * hp + 1
            wqk_np[:, hp, 0:64] = qkv_w[:, h0, 0:64] * scale
            wqk_np[:, hp, 64:128] = qkv_w[:, h1, 0:64] * scale
            wqk_np[:, 4 + hp, 0:64] = qkv_w[:, h0, 64:128]
            wqk_np[:, 4 + hp, 64:128] = qkv_w[:, h1, 64:128]
        wqk_np = np.ascontiguousarray(
            wqk_np.reshape(NET, 128, 8, 128).transpose(1, 0, 2, 3).reshape(128, -1)
        ).astype(bf16)

        wv_np = np.ascontiguousarray(
            qkv_w[:, heads, 128:192].reshape(NET, 128, 512)
            .transpose(1, 0, 2).reshape(128, -1)
        ).astype(bf16)
        ow_np = np.ascontiguousarray(
            out_w[heads].reshape(512, E).reshape(NPAIR, 128, E)
            .transpose(1, 0, 2).reshape(128, -1)
        ).astype(bf16)

        m = {
            "xT": xT_b[b],
            "wqk": wqk_np,
            "wv": wv_np,
            "ow": ow_np,
            "masks": masks_np,
            "cones": cones_np,
            "selq": selq_np,
        }
        if with_bias:
            qb_np = np.empty((1, 8 * 128), np.float32)
            for hp in range(NPAIR):
                h0, h1 = hs + 2 * hp, hs + 2 * hp + 1
                qb_np[0, hp * 128: hp * 128 + 64] = qkv_b[h0, 0:64] * scale
                qb_np[0, hp * 128 + 64: hp * 128 + 128] = qkv_b[h1, 0:64] * scale
                qb_np[0, (4 + hp) * 128: (4 + hp) * 128 + 64] = qkv_b[h0, 64:128]
                qb_np[0, (4 + hp) * 128 + 64: (4 + hp) * 128 + 128] = qkv_b[h1, 64:128]
            vb_np = qkv_b[heads, 128:192].reshape(1, 512).astype(np.float32)
            m["qb"] = qb_np.astype(bf16)
            m["vb"] = vb_np.astype(bf16)
        in_maps.append(m)
    return in_maps, with_bias


# ---------------------------------------------------------------------------
# execution (adapted from bass2jax.run_bass_via_pjrt, with jit reuse)
# ---------------------------------------------------------------------------

_runner_cache = {}


def _get_runner(with_qkv_bias: bool):
    key = with_qkv_bias
    if key in _runner_cache:
        return _runner_cache[key]

    _apply_patches()
    import jax
    from jax.sharding import Mesh, PartitionSpec
    from jax.experimental.shard_map import shard_map
    from concourse import bass2jax
    from concourse import mybir as _mybir

    nc = _build_nc(with_qkv_bias)

    partition_name = nc.partition_id_tensor.name if nc.partition_id_tensor else None
    in_names, out_names, out_avals = [], [], []
    for alloc in nc.m.functions[0].allocations:
        if not isinstance(alloc, _mybir.MemoryLocationSet):
            continue
        name = alloc.memorylocations[0].name
        if alloc.kind == "ExternalInput":
            if name != partition_name:
                in_names.append(name)
        elif alloc.kind == "ExternalOutput":
            out_names.append(name)
            out_avals.append(
                jax.core.ShapedArray(
                    tuple(alloc.tensor_shape), _mybir.dt.np(alloc.dtype)
                )
            )
    n_params = len(in_names)
    all_in_names = in_names + out_names
    if partition_name is not None:
        all_in_names = all_in_names + [partition_name]

    def _body(*args):
        operands = list(args)
        if partition_name is not None:
            operands.append(bass2jax.partition_id_tensor())
        outs = bass2jax._bass_exec_p.bind(
            *operands,
            out_avals=tuple(out_avals),
            in_names=tuple(all_in_names),
            out_names=tuple(out_names),
            lowering_input_output_aliases=(),
            sim_require_finite=True,
            sim_require_nnan=True,
            nc=nc,
        )
        return tuple(outs)

    devices = jax.devices()[:N_CORES]
    mesh = Mesh(np.asarray(devices), ("core",))
    n_outs = len(out_names)
    sharded = jax.jit(
        shard_map(
            _body,
            mesh=mesh,
            in_specs=(PartitionSpec("core"),) * (n_params + n_outs),
            out_specs=(PartitionSpec("core"),) * n_outs,
            check_rep=False,
        ),
        keep_unused=True,
    )

    runner = {
        "fn": sharded,
        "in_names": in_names,
        "out_names": out_names,
        "out_avals": out_avals,
        "n_params": n_params,
    }
    _runner_cache[key] = runner
    return runner


def _run_spmd(runner, in_maps):
    concat_in = [
        np.concatenate([np.asarray(in_maps[c][name]) for c in range(N_CORES)], axis=0)
        for name in runner["in_names"]
    ]
    concat_zeros = [
        np.zeros((N_CORES * a.shape[0], *a.shape[1:]), a.dtype)
        for a in runner["out_avals"]
    ]
    out_arrs = runner["fn"](*concat_in, *concat_zeros)
    name_to_idx = {n: i for i, n in enumerate(runner["out_names"])}
    i = name_to_idx["out"]
    a = runner["out_avals"][i]
    return np.asarray(out_arrs[i]).reshape(N_CORES, *a.shape)


def kernel(x, qkv_w, qkv_b, out_w, out_b):
    in_maps, with_bias = _pack_inputs(x, qkv_w, qkv_b, out_w)
    runner = _get_runner(with_bias)
    outs = _run_spmd(runner, in_maps)  # [8, S, E] partials
    out_b = np.asarray(out_b, np.float32)
    result = np.empty((B, S, E), np.float32)
    for b in range(B):
        result[b] = outs[2 * b] + outs[2 * b + 1] + out_b
    return result
